# revision 11
# baseline (speedup 1.0000x reference)
"""Guided filter (radius=3) on 8x TRN2 NeuronCores, batch-parallel. v2.

Per core: one image. Box filters = banded matmuls on the PE (exactly one
layout-swap pass + one layout-keep pass per separable box; 28 passes/image).

v2 vs baseline:
  - Custom fused DVE ops: SCALE_SUB_SQ (var = S*uII_psum - uI^2, kills the
    separate square), CLAMP01_ADDSC (q = clip(S*(Q1*I + Q2)), kills the
    evac+add+clamp chain -- the combine reads both PSUM keeps directly).
  - Engine rebalance: ACT owns all PSUM bridge evacuations (1.11us/tile),
    Pool owns the bf16 products (I*I, I*p) + S2 B-copies + q DMA issue,
    DVE owns the PSUM-math (var/cov/recip/q-combine) + bf16 chain ops.
  - Stage-2 bridge evacs are pure copies (the 64/49 normalization is folded
    into the final fused clamp), so Pool's copy path can carry them.
  - Channel-pipelined emission: S1 of channel c+1 interleaves with S2 of
    channel c (a/b strip pools sized 15 = 9 + 6 lookahead).
"""

import sys

sys.path.insert(0, "/opt/trn_rl_repo")

import numpy as np
import ml_dtypes

R = 3
H = W = 1024
P = 128
V = 122  # valid outputs per 128-wide band matmul
S = float(64.0 / 49.0)

_cache = {}


# ---------------------------------------------------------------- custom DVE
def _register_ops():
    from concourse.dve_ops import DveOp, OPS, _SUB_OPCODE_FOR_NAME
    from concourse.dve_spec import (
        Spec, Src0, Src1, C0, Zero, One, maxx, minn, sq, lower,
    )
    from concourse.dve_uop import DveOpSpec

    def reg(name, spec):
        existing = {op.name: op for op in OPS}
        if name in existing:
            return existing[name]
        opcode = max(_SUB_OPCODE_FOR_NAME.values()) + 1
        assert opcode < 0x20, "custom-DVE opcode table full"
        shas = {}
        for ver in ("v3", "v4"):
            s = DveOpSpec(name=name, opcode=opcode, uops=lower(spec, ver=ver))
            shas[ver] = s.sha(ver)
        op = DveOp(name, spec, subdim=False, uops_sha=shas)
        OPS.append(op)
        _SUB_OPCODE_FOR_NAME[name] = opcode
        return op

    clamp = reg(
        "CLAMP01_ADDSC_ANT",
        Spec(
            body=minn(maxx((Src0 + Src1) * C0, Zero), One),
            reference=lambda in0, in1, s0, s1, imm2: np.clip(
                (in0 + in1) * s0, 0.0, 1.0
            ),
        ),
    )
    var = reg(
        "SCALE_SUB_SQ_ANT",
        Spec(
            body=Src0 * C0 - sq(Src1),
            reference=lambda in0, in1, s0, s1, imm2: in0 * s0 - in1 * in1,
        ),
    )
    return clamp, var


def _strips():
    # (in_lo, in_hi, out_lo, out_hi) along one axis
    out = []
    j = 0
    while j * V < W:
        o_lo, o_hi = j * V, min(W, j * V + V)
        i_lo, i_hi = max(0, o_lo - R), min(W, o_hi + R)
        out.append((i_lo, i_hi, o_lo, o_hi))
        j += 1
    return out


def _band7_np():
    b = np.zeros((128, 134), np.float32)
    for k in range(128):
        for d in range(134):
            if abs(d - 3 - k) <= R:
                b[k, d] = 0.125
    return b.astype(ml_dtypes.bfloat16)


def _bandm_np(i_lo, i_hi, o_lo, o_hi):
    K = i_hi - i_lo
    bm = np.zeros((K, 128), np.float32)
    for k in range(K):
        for m in range(o_hi - o_lo):
            if abs((i_lo + k) - (o_lo + m)) <= R:
                bm[k, m] = 0.125
    return bm.astype(ml_dtypes.bfloat16)


def _seg512(lo, hi):
    """split [lo,hi) at multiples of 512 (PSUM bank boundaries)"""
    segs = []
    while lo < hi:
        nxt = min(hi, (lo // 512 + 1) * 512)
        segs.append((lo, nxt))
        lo = nxt
    return segs


def _build():
    import concourse.bass as bass
    import concourse.bacc as bacc
    import concourse.mybir as mybir
    from concourse import tile
    from concourse.ap import AP as _AP

    CLAMP01_ADDSC, SCALE_SUB_SQ = _register_ops()

    bf16 = mybir.dt.bfloat16
    f32 = mybir.dt.float32
    f8 = mybir.dt.float8e4
    Copy = mybir.ActivationFunctionType.Copy
    Alu = mybir.AluOpType

    strips = _strips()
    NS = len(strips)

    nc = bacc.Bacc(None, target_bir_lowering=False)
    dI = nc.dram_tensor("I", [H, W], f32, kind="ExternalInput")
    dp = nc.dram_tensor("p", [3, H, W], f32, kind="ExternalInput")
    db7 = nc.dram_tensor("band7", [128, 134], bf16, kind="ExternalInput")
    dbm_f = nc.dram_tensor("bandm_first", [125, 128], bf16, kind="ExternalInput")
    dbm_i = nc.dram_tensor("bandm_int", [128, 128], bf16, kind="ExternalInput")
    dbm_l = nc.dram_tensor("bandm_last", [51, 128], bf16, kind="ExternalInput")
    dq = nc.dram_tensor("q", [3, H, W], f32, kind="ExternalOutput")

    with tile.TileContext(nc) as tc:
        with (
            tc.tile_pool(name="const", bufs=1) as constp,
            tc.tile_pool(name="resid", bufs=1) as residp,
            tc.tile_pool(name="pbuf", bufs=2) as pbufp,
            tc.tile_pool(name="prod", bufs=3) as prodp,
            tc.tile_pool(name="uirv", bufs=1) as uirvp,
            tc.tile_pool(name="ab", bufs=16) as abp,
            tc.tile_pool(name="brg", bufs=4) as brgp,
            tc.tile_pool(name="upv", bufs=2) as upvp,
            tc.tile_pool(name="f32tmp", bufs=2) as f32p,
            tc.tile_pool(name="mtmp", bufs=5) as mtmpp,
            tc.tile_pool(name="qm", bufs=2) as qmp,
            tc.tile_pool(name="psA", bufs=2, space="PSUM") as psA,
            tc.tile_pool(name="psB", bufs=2, space="PSUM") as psB,
        ):
            band7 = constp.tile([128, 134], bf16, tag="band7")
            nc.sync.dma_start(band7[:], db7.ap()[:])
            bm_first = constp.tile([125, 128], bf16, tag="bmf")
            nc.sync.dma_start(bm_first[:], dbm_f.ap()[:])
            bm_int = constp.tile([128, 128], bf16, tag="bmi")
            nc.sync.dma_start(bm_int[:], dbm_i.ap()[:])
            bm_last = constp.tile([51, 128], bf16, tag="bml")
            nc.sync.dma_start(bm_last[:], dbm_l.ap()[:])

            # Resident inputs, cast f32->bf16 during SWDGE DMA. Block layout:
            # tile[pp, i*1024 + w] = X[i*128 + pp, w].
            I_bf = residp.tile([128, 8 * 1024], bf16, tag="I_bf")
            nc.gpsimd.dma_start(
                I_bf[:].rearrange("p (i w) -> p i w", w=1024),
                dI.ap().rearrange("(i p) w -> p i w", p=128),
            )

            p_tiles = {}

            def load_p(c):
                t = pbufp.tile([128, 8 * 1024], bf16, tag="p_bf")
                nc.gpsimd.dma_start(
                    t[:].rearrange("p (i w) -> p i w", w=1024),
                    dp.ap()[c].rearrange("(i p) w -> p i w", p=128),
                )
                p_tiles[c] = t

            load_p(0)

            def load_I_nat():
                # 122-stride window layout for the final combine:
                # I_nat[j, m*1024 + w] = I[122m + j, w]
                t = residp.tile([128, NS * 1024], bf16, tag="I_nat")
                src = dI.ap()
                src_ov = _AP(src.tensor, 0, [[1024, 128], [V * 1024, 8], [1, 1024]])
                nc.gpsimd.dma_start(
                    t[:, 0 : 8 * 1024].rearrange("p (m w) -> p m w", w=1024), src_ov
                )
                nc.gpsimd.dma_start(
                    t[0 : H - 8 * V, 8 * 1024 : 9 * 1024], dI.ap()[8 * V : H, :]
                )
                return t

            def bandm_for(si):
                if si == 0:
                    return bm_first
                if si == NS - 1:
                    return bm_last
                return bm_int

            # ---------------- matmul pass emitters ----------------
            def swap_pass(ps, tile_, off0, stride, Mw):
                """transpose+V-box: image chunks stationary, band streams.
                Accumulates 8 h-blocks into ps[0:Mw, 0:1024]."""
                seen = set()
                for i in range(8):
                    lhsT = tile_[:, i * stride + off0 : i * stride + off0 + Mw]
                    base = 128 * i - 3
                    w_lo_ = max(0, 128 * i - 3)
                    w_hi_ = min(1024, 128 * i + 131)
                    for s_lo, s_hi in _seg512(w_lo_, w_hi_):
                        bank = s_lo // 512
                        nc.tensor.matmul(
                            ps[0:Mw, s_lo:s_hi],
                            lhsT,
                            band7[:, s_lo - base : s_hi - base],
                            start=bank not in seen,
                            stop=True,
                        )
                        seen.add(bank)

            def keep_pass(ps, bridge, si):
                """H-box, band stationary: ps[0:K_out, 0:1024]"""
                i_lo, i_hi, o_lo, o_hi = strips[si]
                K = i_hi - i_lo
                bm = bandm_for(si)
                for s_lo, s_hi in _seg512(0, 1024):
                    nc.tensor.matmul(
                        ps[:, s_lo:s_hi],
                        bm[:],
                        bridge[0:K, s_lo:s_hi],
                        start=True,
                        stop=True,
                    )

            def b1_pass(ps, tiles_, m_lo, m_hi, band=None):
                """H-box of a/b over w'-strips; out ps[0:(m_hi-m_lo), 0:1024] (N)"""
                seen = set()
                for sj, (ji_lo, ji_hi, jo_lo, jo_hi) in enumerate(strips):
                    K = jo_hi - jo_lo
                    lhsT = tiles_[sj][0:K, m_lo:m_hi]
                    base = jo_lo - 3
                    w_lo_ = max(0, jo_lo - 3)
                    w_hi_ = min(1024, jo_lo + 125)
                    for s_lo, s_hi in _seg512(w_lo_, w_hi_):
                        bank = s_lo // 512
                        nc.tensor.matmul(
                            ps[0 : m_hi - m_lo, s_lo:s_hi],
                            lhsT,
                            (band if band is not None else band7)[0:K, s_lo - base : s_hi - base],
                            start=bank not in seen,
                            stop=True,
                        )
                        seen.add(bank)

            I3 = I_bf[:].rearrange("p (i w) -> p i w", w=1024)

            uI_T = uirvp.tile([128, NS * 1024], bf16, tag="uI_T")
            rv_T = uirvp.tile([128, NS * 1024], bf16, tag="rv_T")

            # products: prefetched one strip ahead, on Pool
            def emit_prod_ii(s):
                i_lo, i_hi, _, _ = strips[s]
                Mw = i_hi - i_lo
                t = prodp.tile([128, 8 * 134], bf16, tag="prod")
                nc.gpsimd.tensor_mul(
                    t[:, 0 : 8 * Mw].rearrange("p (i w) -> p i w", w=Mw),
                    I3[:, :, i_lo:i_hi],
                    I3[:, :, i_lo:i_hi],
                )
                return t

            def emit_prod_ip(c, s):
                i_lo, i_hi, _, _ = strips[s]
                Mw = i_hi - i_lo
                p3 = p_tiles[c][:].rearrange("p (i w) -> p i w", w=1024)
                t = prodp.tile([128, 8 * 134], bf16, tag="prod")
                nc.gpsimd.tensor_mul(
                    t[:, 0 : 8 * Mw].rearrange("p (i w) -> p i w", w=Mw),
                    I3[:, :, i_lo:i_hi],
                    p3[:, :, i_lo:i_hi],
                )
                return t

            # ------- elementwise tails (SBUF-only, engine-flexible) -------
            def s1_tail(c, s, psu, psu2, a_tiles, b_tiles):
                """up evac + a/b chain for strip s, channel c."""
                i_lo, i_hi, o_lo, o_hi = strips[s]
                K_out = o_hi - o_lo
                uI = uI_T[:, s * 1024 : (s + 1) * 1024]
                rv = rv_T[:, s * 1024 : (s + 1) * 1024]
                up = upvp.tile([128, 1024], bf16, tag="up")
                nc.scalar.activation(up[0:K_out, :], psu[0:K_out, :], Copy, bias=0.0, scale=S)
                m1 = mtmpp.tile([128, 1024], bf16, tag="mt")
                nc.vector.tensor_mul(m1[0:K_out, :], uI[0:K_out, :], up[0:K_out, :])
                cov = mtmpp.tile([128, 1024], bf16, tag="mt")
                nc.vector.scalar_tensor_tensor(
                    cov[0:K_out, :], psu2[0:K_out, :], S, m1[0:K_out, :],
                    Alu.mult, Alu.subtract,
                )
                a_t = abp.tile([128, 1024], bf16, tag="a")
                nc.vector.tensor_mul(a_t[0:K_out, :], cov[0:K_out, :], rv[0:K_out, :])
                m2 = mtmpp.tile([128, 1024], bf16, tag="mt")
                nc.gpsimd.tensor_mul(m2[0:K_out, :], a_t[0:K_out, :], uI[0:K_out, :])
                b_t = abp.tile([128, 1024], bf16, tag="b")
                nc.vector.tensor_sub(b_t[0:K_out, :], up[0:K_out, :], m2[0:K_out, :])
                a_tiles.append(a_t)
                b_tiles.append(b_t)

            def s2_combine(c, m, psd_a, psd_b):
                mi_lo, mi_hi, mo_lo, mo_hi = strips[m]
                Hw = mo_hi - mo_lo
                q1 = mtmpp.tile([128, 1024], bf16, tag="mt")
                nc.vector.tensor_mul(
                    q1[0:Hw, :], psd_a[0:Hw, :], I_nat[0:Hw, m * 1024 : (m + 1) * 1024]
                )
                q_t = qmp.tile([128, 1024], bf16, tag="qm")
                nc.vector._custom_dve(
                    CLAMP01_ADDSC,
                    out=q_t[0:Hw, :],
                    in0=q1[0:Hw, :],
                    in1=psd_b[0:Hw, :],
                    s0=S,
                )
                nc.gpsimd.dma_start(dq.ap()[c][mo_lo:mo_hi, :], q_t[0:Hw, :])

            def evac(ps, rows, scale=1.0):
                t = brgp.tile([128, 1024], bf16, tag="brg")
                nc.scalar.activation(t[0:rows, :], ps[0:rows, :], Copy, bias=0.0, scale=scale)
                return t

            # -------- fused iteration: S2(c,m) micro-interleaved with S1(c1,s)
            # PE always has independent work between dependent steps.
            def iter_fused(s2=None, s1=None, prod_next=None):
                # s2 = (c, m, a_tiles, b_tiles); s1 = (c1, s, ip_t, a_next, b_next)
                if s2 is not None:
                    c, m, a_tiles, b_tiles = s2
                    mi_lo, mi_hi, mo_lo, mo_hi = strips[m]
                    Mi = mi_hi - mi_lo
                    psc_a = psA.tile([128, 1024], f32, tag="psa")
                    b1_pass(psc_a, a_tiles, mi_lo, mi_hi)
                    psc_b = psA.tile([128, 1024], f32, tag="psa")
                    b1_pass(psc_b, b_tiles, mi_lo, mi_hi)
                    A_t = evac(psc_a, Mi)
                    B_t = evac(psc_b, Mi)
                if prod_next is not None:
                    prod_next()
                if s1 is not None:
                    c1, s, ip_t, a_next, b_next = s1
                    i_lo, i_hi, o_lo, o_hi = strips[s]
                    Mw = i_hi - i_lo
                    psa = psA.tile([128, 1024], f32, tag="psa")
                    swap_pass(psa, p_tiles[c1], i_lo, 1024, Mw)
                    psa2 = psA.tile([128, 1024], f32, tag="psa")
                    swap_pass(psa2, ip_t, 0, Mw, Mw)
                if s2 is not None:
                    psd_a = psB.tile([128, 1024], f32, tag="psb")
                    keep_pass(psd_a, A_t, m)
                    psd_b = psB.tile([128, 1024], f32, tag="psb")
                    keep_pass(psd_b, B_t, m)
                if s1 is not None:
                    v1 = evac(psa, Mw)
                    v2 = evac(psa2, Mw)
                if s2 is not None:
                    s2_combine(c, m, psd_a, psd_b)
                if s1 is not None:
                    psu = psB.tile([128, 1024], f32, tag="psb")
                    keep_pass(psu, v1, s)
                    psu2 = psB.tile([128, 1024], f32, tag="psb")
                    keep_pass(psu2, v2, s)
                    s1_tail(c1, s, psu, psu2, a_next, b_next)

            # -------- phase-1 strip block: I/II + p0/Ip0 interleaved
            def iter_phase1(s, ii_t, ip_t):
                i_lo, i_hi, o_lo, o_hi = strips[s]
                Mw = i_hi - i_lo
                K_out = o_hi - o_lo
                psa = psA.tile([128, 1024], f32, tag="psa")
                swap_pass(psa, I_bf, i_lo, 1024, Mw)
                psa2 = psA.tile([128, 1024], f32, tag="psa")
                swap_pass(psa2, ii_t, 0, Mw, Mw)
                v1 = evac(psa, Mw)
                v2 = evac(psa2, Mw)
                nxt = []
                if s + 1 < NS:
                    nxt = [emit_prod_ii(s + 1), emit_prod_ip(0, s + 1)]
                psu = psB.tile([128, 1024], f32, tag="psb")
                keep_pass(psu, v1, s)
                psu2 = psB.tile([128, 1024], f32, tag="psb")
                keep_pass(psu2, v2, s)
                psa3 = psA.tile([128, 1024], f32, tag="psa")
                swap_pass(psa3, p_tiles[0], i_lo, 1024, Mw)
                psa4 = psA.tile([128, 1024], f32, tag="psa")
                swap_pass(psa4, ip_t, 0, Mw, Mw)
                uI = uI_T[:, s * 1024 : (s + 1) * 1024]
                nc.scalar.activation(uI[0:K_out, :], psu[0:K_out, :], Copy, bias=0.0, scale=S)
                var_t = f32p.tile([128, 1024], f32, tag="f32")
                nc.vector._custom_dve(
                    SCALE_SUB_SQ,
                    out=var_t[0:K_out, :],
                    in0=psu2[0:K_out, :],
                    in1=uI[0:K_out, :],
                    s0=S,
                )
                v3 = evac(psa3, Mw)
                v4 = evac(psa4, Mw)
                rv32 = f32p.tile([128, 1024], f32, tag="f32")
                nc.vector.reciprocal_approx_fast(rv32[0:K_out, :], var_t[0:K_out, :])
                psu3 = psB.tile([128, 1024], f32, tag="psb")
                keep_pass(psu3, v3, s)
                psu4 = psB.tile([128, 1024], f32, tag="psb")
                keep_pass(psu4, v4, s)
                rv = rv_T[:, s * 1024 : (s + 1) * 1024]
                nc.scalar.activation(rv[0:K_out, :], rv32[0:K_out, :], Copy, bias=0.0, scale=1.0)
                return nxt, psu3, psu4

            # ---------------- emission schedule ----------------
            LOOKAHEAD = 7  # ab pool = 9 + 7

            with nc.named_scope("phase1"):
                ii_t = emit_prod_ii(0)
                ip_t = emit_prod_ip(0, 0)
                I_nat = load_I_nat()
                load_p(1)
                ab0_a, ab0_b = [], []
                for s in range(NS):
                    nxt, psu3, psu4 = iter_phase1(s, ii_t, ip_t)
                    s1_tail(0, s, psu3, psu4, ab0_a, ab0_b)
                    if nxt:
                        ii_t, ip_t = nxt

            ab_saved = {0: (ab0_a, ab0_b)}
            for c in (0, 1):
                with nc.named_scope(f"c{c}"):
                    if c == 0:
                        load_p(2)
                    a_next, b_next = [], []
                    state = {"t": emit_prod_ip(c + 1, 0)}

                    def mk_prod(sn):
                        def f():
                            state["t2"] = emit_prod_ip(c + 1, sn + 1)
                        return f if sn + 1 < NS else None

                    for m in range(NS):
                        sn = m - (NS - 1 - LOOKAHEAD)
                        if 0 <= sn < LOOKAHEAD:
                            iter_fused(
                                s2=(c, m, *ab_saved[c]),
                                s1=(c + 1, sn, state["t"], a_next, b_next),
                                prod_next=mk_prod(sn),
                            )
                            state["t"] = state.get("t2")
                        else:
                            iter_fused(s2=(c, m, *ab_saved[c]))
                    for sn in range(LOOKAHEAD, NS):
                        iter_fused(
                            s1=(c + 1, sn, state["t"], a_next, b_next),
                            prod_next=mk_prod(sn),
                        )
                        state["t"] = state.get("t2")
                    ab_saved[c + 1] = (a_next, b_next)

            # c2: software-pipeline S2 blocks 2-deep (b1s of m+1 before keeps of m)
            with nc.named_scope("c2"):
                a2, b2 = ab_saved[2]
                pend = None  # (m, A_t, B_t)
                for m in range(NS):
                    mi_lo, mi_hi, mo_lo, mo_hi = strips[m]
                    Mi = mi_hi - mi_lo
                    psc_a = psA.tile([128, 1024], f32, tag="psa")
                    b1_pass(psc_a, a2, mi_lo, mi_hi)
                    psc_b = psA.tile([128, 1024], f32, tag="psa")
                    b1_pass(psc_b, b2, mi_lo, mi_hi)
                    A_t = evac(psc_a, Mi)
                    B_t = evac(psc_b, Mi)
                    if pend is not None:
                        pm, pA, pB = pend
                        psd_a = psB.tile([128, 1024], f32, tag="psb")
                        keep_pass(psd_a, pA, pm)
                        psd_b = psB.tile([128, 1024], f32, tag="psb")
                        keep_pass(psd_b, pB, pm)
                        s2_combine(2, pm, psd_a, psd_b)
                    pend = (m, A_t, B_t)
                pm, pA, pB = pend
                psd_a = psB.tile([128, 1024], f32, tag="psb")
                keep_pass(psd_a, pA, pm)
                psd_b = psB.tile([128, 1024], f32, tag="psb")
                keep_pass(psd_b, pB, pm)
                s2_combine(2, pm, psd_a, psd_b)

    nc.compile()
    return nc


def kernel(I, p, radius):
    assert int(radius) == R
    I = np.ascontiguousarray(np.asarray(I, np.float32))
    p = np.ascontiguousarray(np.asarray(p, np.float32))
    B = I.shape[0]
    assert I.shape == (B, 1, H, W) and p.shape == (B, 3, H, W)

    if "nc" not in _cache:
        _cache["nc"] = _build()
    nc = _cache["nc"]

    from concourse.bass_utils import run_bass_kernel_spmd

    b7 = _band7_np()
    strips = _strips()
    bm_f = _bandm_np(*strips[0])
    bm_i = _bandm_np(*strips[1])
    bm_l = _bandm_np(*strips[-1])

    in_maps = []
    for i in range(B):
        in_maps.append(
            {
                "I": I[i, 0],
                "p": p[i],
                "band7": b7,
                "bandm_first": bm_f,
                "bandm_int": bm_i,
                "bandm_last": bm_l,
            }
        )
    res = run_bass_kernel_spmd(nc, in_maps, core_ids=list(range(B)))
    out = np.stack([res.results[i]["q"] for i in range(B)], axis=0)
    return out.astype(np.float32)


# revision 12
# speedup vs baseline: 1.0654x; 1.0654x over previous
"""Guided filter (radius=3) on 8x TRN2 NeuronCores, batch-parallel.

Per core: one image. Box filters = banded matmuls on the PE (exactly one
layout-swap pass + one layout-keep pass per separable box; 28 passes/image).

vs the original baseline:
  - Custom fused DVE ops: SCALE_SUB_SQ (var = S*uII_psum - uI^2, kills the
    separate square), CLAMP01_ADDSC (q = clip(S*(Q1*I + Q2)) reads both
    stage-2 PSUM keeps directly, killing two evacs + add + clamp per tile).
  - Engine rebalance: ACT owns all PSUM bridge evacuations, Pool owns the
    bf16 products (I*I, I*p) + q DMA issue, DVE owns PSUM-math
    (var/cov/recip/combine) + the bf16 a/b chain.
  - Stage-2 bridge evacs are pure copies (64/49 normalization folded into
    the final fused clamp).
  - Micro-interleaved emission: each iteration interleaves S2(c, m) with
    S1(c+1, strip) step-by-step so PE always has independent work between
    dependent stages; c2's S2 blocks are software-pipelined 2-deep.
  - Startup DMA order: I_bf + p0 first, I_nat/p1 after the first products.
"""

import sys

sys.path.insert(0, "/opt/trn_rl_repo")

import numpy as np
import ml_dtypes

R = 3
H = W = 1024
P = 128
V = 122  # valid outputs per 128-wide band matmul
S = float(64.0 / 49.0)

_cache = {}


# ---------------------------------------------------------------- custom DVE
def _register_ops():
    from concourse.dve_ops import DveOp, OPS, _SUB_OPCODE_FOR_NAME
    from concourse.dve_spec import (
        Spec, Src0, Src1, C0, Zero, One, maxx, minn, sq, lower,
    )
    from concourse.dve_uop import DveOpSpec

    def reg(name, spec):
        existing = {op.name: op for op in OPS}
        if name in existing:
            return existing[name]
        opcode = max(_SUB_OPCODE_FOR_NAME.values()) + 1
        assert opcode < 0x20, "custom-DVE opcode table full"
        shas = {}
        for ver in ("v3", "v4"):
            s = DveOpSpec(name=name, opcode=opcode, uops=lower(spec, ver=ver))
            shas[ver] = s.sha(ver)
        op = DveOp(name, spec, subdim=False, uops_sha=shas)
        OPS.append(op)
        _SUB_OPCODE_FOR_NAME[name] = opcode
        return op

    clamp = reg(
        "CLAMP01_ADDSC_ANT",
        Spec(
            body=minn(maxx((Src0 + Src1) * C0, Zero), One),
            reference=lambda in0, in1, s0, s1, imm2: np.clip(
                (in0 + in1) * s0, 0.0, 1.0
            ),
        ),
    )
    var = reg(
        "SCALE_SUB_SQ_ANT",
        Spec(
            body=Src0 * C0 - sq(Src1),
            reference=lambda in0, in1, s0, s1, imm2: in0 * s0 - in1 * in1,
        ),
    )
    return clamp, var


def _strips():
    # (in_lo, in_hi, out_lo, out_hi) along one axis
    out = []
    j = 0
    while j * V < W:
        o_lo, o_hi = j * V, min(W, j * V + V)
        i_lo, i_hi = max(0, o_lo - R), min(W, o_hi + R)
        out.append((i_lo, i_hi, o_lo, o_hi))
        j += 1
    return out


def _band7_np():
    b = np.zeros((128, 134), np.float32)
    for k in range(128):
        for d in range(134):
            if abs(d - 3 - k) <= R:
                b[k, d] = 0.125
    return b.astype(ml_dtypes.bfloat16)


def _bandm_np(i_lo, i_hi, o_lo, o_hi):
    K = i_hi - i_lo
    bm = np.zeros((K, 128), np.float32)
    for k in range(K):
        for m in range(o_hi - o_lo):
            if abs((i_lo + k) - (o_lo + m)) <= R:
                bm[k, m] = 0.125
    return bm.astype(ml_dtypes.bfloat16)


def _seg512(lo, hi):
    """split [lo,hi) at multiples of 512 (PSUM bank boundaries)"""
    segs = []
    while lo < hi:
        nxt = min(hi, (lo // 512 + 1) * 512)
        segs.append((lo, nxt))
        lo = nxt
    return segs


def _build():
    import concourse.bass as bass
    import concourse.bacc as bacc
    import concourse.mybir as mybir
    from concourse import tile
    from concourse.ap import AP as _AP

    CLAMP01_ADDSC, SCALE_SUB_SQ = _register_ops()

    bf16 = mybir.dt.bfloat16
    f32 = mybir.dt.float32
    f8 = mybir.dt.float8e4
    Copy = mybir.ActivationFunctionType.Copy
    Alu = mybir.AluOpType

    strips = _strips()
    NS = len(strips)

    nc = bacc.Bacc(None, target_bir_lowering=False)
    dI = nc.dram_tensor("I", [H, W], f32, kind="ExternalInput")
    dp = nc.dram_tensor("p", [3, H, W], f32, kind="ExternalInput")
    db7 = nc.dram_tensor("band7", [128, 134], bf16, kind="ExternalInput")
    dbm_f = nc.dram_tensor("bandm_first", [125, 128], bf16, kind="ExternalInput")
    dbm_i = nc.dram_tensor("bandm_int", [128, 128], bf16, kind="ExternalInput")
    dbm_l = nc.dram_tensor("bandm_last", [51, 128], bf16, kind="ExternalInput")
    dq = nc.dram_tensor("q", [3, H, W], f32, kind="ExternalOutput")

    with tile.TileContext(nc) as tc:
        with (
            tc.tile_pool(name="const", bufs=1) as constp,
            tc.tile_pool(name="resid", bufs=1) as residp,
            tc.tile_pool(name="pbuf", bufs=2) as pbufp,
            tc.tile_pool(name="prod", bufs=3) as prodp,
            tc.tile_pool(name="uirv", bufs=1) as uirvp,
            tc.tile_pool(name="ab", bufs=16) as abp,
            tc.tile_pool(name="brg", bufs=4) as brgp,
            tc.tile_pool(name="upv", bufs=2) as upvp,
            tc.tile_pool(name="f32tmp", bufs=2) as f32p,
            tc.tile_pool(name="mtmp", bufs=5) as mtmpp,
            tc.tile_pool(name="qm", bufs=2) as qmp,
            tc.tile_pool(name="psA", bufs=2, space="PSUM") as psA,
            tc.tile_pool(name="psB", bufs=2, space="PSUM") as psB,
        ):
            band7 = constp.tile([128, 134], bf16, tag="band7")
            nc.sync.dma_start(band7[:], db7.ap()[:])
            bm_first = constp.tile([125, 128], bf16, tag="bmf")
            nc.sync.dma_start(bm_first[:], dbm_f.ap()[:])
            bm_int = constp.tile([128, 128], bf16, tag="bmi")
            nc.sync.dma_start(bm_int[:], dbm_i.ap()[:])
            bm_last = constp.tile([51, 128], bf16, tag="bml")
            nc.sync.dma_start(bm_last[:], dbm_l.ap()[:])

            # Resident inputs, cast f32->bf16 during SWDGE DMA. Block layout:
            # tile[pp, i*1024 + w] = X[i*128 + pp, w].
            I_bf = residp.tile([128, 8 * 1024], bf16, tag="I_bf")
            nc.gpsimd.dma_start(
                I_bf[:].rearrange("p (i w) -> p i w", w=1024),
                dI.ap().rearrange("(i p) w -> p i w", p=128),
            )

            p_tiles = {}

            def load_p(c):
                t = pbufp.tile([128, 8 * 1024], bf16, tag="p_bf")
                nc.gpsimd.dma_start(
                    t[:].rearrange("p (i w) -> p i w", w=1024),
                    dp.ap()[c].rearrange("(i p) w -> p i w", p=128),
                )
                p_tiles[c] = t

            load_p(0)

            def load_I_nat():
                # 122-stride window layout for the final combine:
                # I_nat[j, m*1024 + w] = I[122m + j, w]
                t = residp.tile([128, NS * 1024], bf16, tag="I_nat")
                src = dI.ap()
                src_ov = _AP(src.tensor, 0, [[1024, 128], [V * 1024, 8], [1, 1024]])
                nc.gpsimd.dma_start(
                    t[:, 0 : 8 * 1024].rearrange("p (m w) -> p m w", w=1024), src_ov
                )
                nc.gpsimd.dma_start(
                    t[0 : H - 8 * V, 8 * 1024 : 9 * 1024], dI.ap()[8 * V : H, :]
                )
                return t

            def bandm_for(si):
                if si == 0:
                    return bm_first
                if si == NS - 1:
                    return bm_last
                return bm_int

            # ---------------- matmul pass emitters ----------------
            def swap_pass(ps, tile_, off0, stride, Mw):
                """transpose+V-box: image chunks stationary, band streams.
                Accumulates 8 h-blocks into ps[0:Mw, 0:1024]."""
                seen = set()
                for i in range(8):
                    lhsT = tile_[:, i * stride + off0 : i * stride + off0 + Mw]
                    base = 128 * i - 3
                    w_lo_ = max(0, 128 * i - 3)
                    w_hi_ = min(1024, 128 * i + 131)
                    for s_lo, s_hi in _seg512(w_lo_, w_hi_):
                        bank = s_lo // 512
                        nc.tensor.matmul(
                            ps[0:Mw, s_lo:s_hi],
                            lhsT,
                            band7[:, s_lo - base : s_hi - base],
                            start=bank not in seen,
                            stop=True,
                        )
                        seen.add(bank)

            def keep_pass(ps, bridge, si):
                """H-box, band stationary: ps[0:K_out, 0:1024]"""
                i_lo, i_hi, o_lo, o_hi = strips[si]
                K = i_hi - i_lo
                bm = bandm_for(si)
                for s_lo, s_hi in _seg512(0, 1024):
                    nc.tensor.matmul(
                        ps[:, s_lo:s_hi],
                        bm[:],
                        bridge[0:K, s_lo:s_hi],
                        start=True,
                        stop=True,
                    )

            def b1_pass(ps, tiles_, m_lo, m_hi, band=None):
                """H-box of a/b over w'-strips; out ps[0:(m_hi-m_lo), 0:1024] (N)"""
                seen = set()
                for sj, (ji_lo, ji_hi, jo_lo, jo_hi) in enumerate(strips):
                    K = jo_hi - jo_lo
                    lhsT = tiles_[sj][0:K, m_lo:m_hi]
                    base = jo_lo - 3
                    w_lo_ = max(0, jo_lo - 3)
                    w_hi_ = min(1024, jo_lo + 125)
                    for s_lo, s_hi in _seg512(w_lo_, w_hi_):
                        bank = s_lo // 512
                        nc.tensor.matmul(
                            ps[0 : m_hi - m_lo, s_lo:s_hi],
                            lhsT,
                            (band if band is not None else band7)[0:K, s_lo - base : s_hi - base],
                            start=bank not in seen,
                            stop=True,
                        )
                        seen.add(bank)

            I3 = I_bf[:].rearrange("p (i w) -> p i w", w=1024)

            uI_T = uirvp.tile([128, NS * 1024], bf16, tag="uI_T")
            rv_T = uirvp.tile([128, NS * 1024], bf16, tag="rv_T")

            # products: prefetched one strip ahead, on Pool
            def emit_prod_ii(s):
                i_lo, i_hi, _, _ = strips[s]
                Mw = i_hi - i_lo
                t = prodp.tile([128, 8 * 134], bf16, tag="prod")
                nc.gpsimd.tensor_mul(
                    t[:, 0 : 8 * Mw].rearrange("p (i w) -> p i w", w=Mw),
                    I3[:, :, i_lo:i_hi],
                    I3[:, :, i_lo:i_hi],
                )
                return t

            def emit_prod_ip(c, s):
                i_lo, i_hi, _, _ = strips[s]
                Mw = i_hi - i_lo
                p3 = p_tiles[c][:].rearrange("p (i w) -> p i w", w=1024)
                t = prodp.tile([128, 8 * 134], bf16, tag="prod")
                nc.gpsimd.tensor_mul(
                    t[:, 0 : 8 * Mw].rearrange("p (i w) -> p i w", w=Mw),
                    I3[:, :, i_lo:i_hi],
                    p3[:, :, i_lo:i_hi],
                )
                return t

            # ------- elementwise tails (SBUF-only, engine-flexible) -------
            def s1_tail(c, s, psu, psu2, a_tiles, b_tiles):
                """up evac + a/b chain for strip s, channel c."""
                i_lo, i_hi, o_lo, o_hi = strips[s]
                K_out = o_hi - o_lo
                uI = uI_T[:, s * 1024 : (s + 1) * 1024]
                rv = rv_T[:, s * 1024 : (s + 1) * 1024]
                up = upvp.tile([128, 1024], bf16, tag="up")
                nc.scalar.activation(up[0:K_out, :], psu[0:K_out, :], Copy, bias=0.0, scale=S)
                m1 = mtmpp.tile([128, 1024], bf16, tag="mt")
                nc.vector.tensor_mul(m1[0:K_out, :], uI[0:K_out, :], up[0:K_out, :])
                cov = mtmpp.tile([128, 1024], bf16, tag="mt")
                nc.vector.scalar_tensor_tensor(
                    cov[0:K_out, :], psu2[0:K_out, :], S, m1[0:K_out, :],
                    Alu.mult, Alu.subtract,
                )
                a_t = abp.tile([128, 1024], bf16, tag="a")
                nc.vector.tensor_mul(a_t[0:K_out, :], cov[0:K_out, :], rv[0:K_out, :])
                m2 = mtmpp.tile([128, 1024], bf16, tag="mt")
                nc.vector.tensor_mul(m2[0:K_out, :], a_t[0:K_out, :], uI[0:K_out, :])
                b_t = abp.tile([128, 1024], bf16, tag="b")
                nc.vector.tensor_sub(b_t[0:K_out, :], up[0:K_out, :], m2[0:K_out, :])
                a_tiles.append(a_t)
                b_tiles.append(b_t)

            def s2_combine(c, m, psd_a, psd_b):
                mi_lo, mi_hi, mo_lo, mo_hi = strips[m]
                Hw = mo_hi - mo_lo
                q1 = mtmpp.tile([128, 1024], bf16, tag="mt")
                nc.vector.tensor_mul(
                    q1[0:Hw, :], psd_a[0:Hw, :], I_nat[0:Hw, m * 1024 : (m + 1) * 1024]
                )
                q_t = qmp.tile([128, 1024], bf16, tag="qm")
                nc.vector._custom_dve(
                    CLAMP01_ADDSC,
                    out=q_t[0:Hw, :],
                    in0=q1[0:Hw, :],
                    in1=psd_b[0:Hw, :],
                    s0=S,
                )
                nc.gpsimd.dma_start(dq.ap()[c][mo_lo:mo_hi, :], q_t[0:Hw, :])

            def evac(ps, rows, scale=1.0):
                t = brgp.tile([128, 1024], bf16, tag="brg")
                nc.scalar.activation(t[0:rows, :], ps[0:rows, :], Copy, bias=0.0, scale=scale)
                return t

            # -------- fused iteration: S2(c,m) micro-interleaved with S1(c1,s)
            # PE always has independent work between dependent steps.
            def iter_fused(s2=None, s1=None, prod_next=None):
                # s2 = (c, m, a_tiles, b_tiles); s1 = (c1, s, ip_t, a_next, b_next)
                if s2 is not None:
                    c, m, a_tiles, b_tiles = s2
                    mi_lo, mi_hi, mo_lo, mo_hi = strips[m]
                    Mi = mi_hi - mi_lo
                    psc_a = psA.tile([128, 1024], f32, tag="psa")
                    b1_pass(psc_a, a_tiles, mi_lo, mi_hi)
                    psc_b = psA.tile([128, 1024], f32, tag="psa")
                    b1_pass(psc_b, b_tiles, mi_lo, mi_hi)
                    A_t = evac(psc_a, Mi)
                    B_t = evac(psc_b, Mi)
                if prod_next is not None:
                    prod_next()
                if s1 is not None:
                    c1, s, ip_t, a_next, b_next = s1
                    i_lo, i_hi, o_lo, o_hi = strips[s]
                    Mw = i_hi - i_lo
                    psa = psA.tile([128, 1024], f32, tag="psa")
                    swap_pass(psa, p_tiles[c1], i_lo, 1024, Mw)
                    psa2 = psA.tile([128, 1024], f32, tag="psa")
                    swap_pass(psa2, ip_t, 0, Mw, Mw)
                if s2 is not None:
                    psd_a = psB.tile([128, 1024], f32, tag="psb")
                    keep_pass(psd_a, A_t, m)
                    psd_b = psB.tile([128, 1024], f32, tag="psb")
                    keep_pass(psd_b, B_t, m)
                if s1 is not None:
                    v1 = evac(psa, Mw)
                    v2 = evac(psa2, Mw)
                if s2 is not None:
                    s2_combine(c, m, psd_a, psd_b)
                if s1 is not None:
                    psu = psB.tile([128, 1024], f32, tag="psb")
                    keep_pass(psu, v1, s)
                    psu2 = psB.tile([128, 1024], f32, tag="psb")
                    keep_pass(psu2, v2, s)
                    s1_tail(c1, s, psu, psu2, a_next, b_next)

            # -------- phase-1 strip block: I/II + p0/Ip0 interleaved
            def iter_phase1(s, ii_t, ip_t):
                i_lo, i_hi, o_lo, o_hi = strips[s]
                Mw = i_hi - i_lo
                K_out = o_hi - o_lo
                psa = psA.tile([128, 1024], f32, tag="psa")
                swap_pass(psa, I_bf, i_lo, 1024, Mw)
                psa2 = psA.tile([128, 1024], f32, tag="psa")
                swap_pass(psa2, ii_t, 0, Mw, Mw)
                v1 = evac(psa, Mw)
                v2 = evac(psa2, Mw)
                nxt = []
                if s + 1 < NS:
                    nxt = [emit_prod_ii(s + 1), emit_prod_ip(0, s + 1)]
                psu = psB.tile([128, 1024], f32, tag="psb")
                keep_pass(psu, v1, s)
                psu2 = psB.tile([128, 1024], f32, tag="psb")
                keep_pass(psu2, v2, s)
                psa3 = psA.tile([128, 1024], f32, tag="psa")
                swap_pass(psa3, p_tiles[0], i_lo, 1024, Mw)
                psa4 = psA.tile([128, 1024], f32, tag="psa")
                swap_pass(psa4, ip_t, 0, Mw, Mw)
                uI = uI_T[:, s * 1024 : (s + 1) * 1024]
                nc.scalar.activation(uI[0:K_out, :], psu[0:K_out, :], Copy, bias=0.0, scale=S)
                var_t = f32p.tile([128, 1024], f32, tag="f32")
                nc.vector._custom_dve(
                    SCALE_SUB_SQ,
                    out=var_t[0:K_out, :],
                    in0=psu2[0:K_out, :],
                    in1=uI[0:K_out, :],
                    s0=S,
                )
                v3 = evac(psa3, Mw)
                v4 = evac(psa4, Mw)
                rv32 = f32p.tile([128, 1024], f32, tag="f32")
                nc.vector.reciprocal_approx_fast(rv32[0:K_out, :], var_t[0:K_out, :])
                psu3 = psB.tile([128, 1024], f32, tag="psb")
                keep_pass(psu3, v3, s)
                psu4 = psB.tile([128, 1024], f32, tag="psb")
                keep_pass(psu4, v4, s)
                rv = rv_T[:, s * 1024 : (s + 1) * 1024]
                nc.scalar.activation(rv[0:K_out, :], rv32[0:K_out, :], Copy, bias=0.0, scale=1.0)
                return nxt, psu3, psu4

            # ---------------- emission schedule ----------------
            LOOKAHEAD = 7  # ab pool = 9 + 7

            with nc.named_scope("phase1"):
                ii_t = emit_prod_ii(0)
                ip_t = emit_prod_ip(0, 0)
                I_nat = load_I_nat()
                load_p(1)
                ab0_a, ab0_b = [], []
                for s in range(NS):
                    nxt, psu3, psu4 = iter_phase1(s, ii_t, ip_t)
                    s1_tail(0, s, psu3, psu4, ab0_a, ab0_b)
                    if nxt:
                        ii_t, ip_t = nxt

            ab_saved = {0: (ab0_a, ab0_b)}
            for c in (0, 1):
                with nc.named_scope(f"c{c}"):
                    if c == 0:
                        load_p(2)
                    a_next, b_next = [], []
                    state = {"t": emit_prod_ip(c + 1, 0)}

                    def mk_prod(sn):
                        def f():
                            state["t2"] = emit_prod_ip(c + 1, sn + 1)
                        return f if sn + 1 < NS else None

                    for m in range(NS):
                        sn = m - (NS - 1 - LOOKAHEAD)
                        if 0 <= sn < LOOKAHEAD:
                            iter_fused(
                                s2=(c, m, *ab_saved[c]),
                                s1=(c + 1, sn, state["t"], a_next, b_next),
                                prod_next=mk_prod(sn),
                            )
                            state["t"] = state.get("t2")
                        else:
                            iter_fused(s2=(c, m, *ab_saved[c]))
                    for sn in range(LOOKAHEAD, NS):
                        iter_fused(
                            s1=(c + 1, sn, state["t"], a_next, b_next),
                            prod_next=mk_prod(sn),
                        )
                        state["t"] = state.get("t2")
                    ab_saved[c + 1] = (a_next, b_next)

            # c2: software-pipeline S2 blocks 2-deep (b1s of m+1 before keeps of m)
            with nc.named_scope("c2"):
                a2, b2 = ab_saved[2]
                pend = None  # (m, A_t, B_t)
                for m in range(NS):
                    mi_lo, mi_hi, mo_lo, mo_hi = strips[m]
                    Mi = mi_hi - mi_lo
                    psc_a = psA.tile([128, 1024], f32, tag="psa")
                    b1_pass(psc_a, a2, mi_lo, mi_hi)
                    psc_b = psA.tile([128, 1024], f32, tag="psa")
                    b1_pass(psc_b, b2, mi_lo, mi_hi)
                    A_t = evac(psc_a, Mi)
                    B_t = evac(psc_b, Mi)
                    if pend is not None:
                        pm, pA, pB = pend
                        psd_a = psB.tile([128, 1024], f32, tag="psb")
                        keep_pass(psd_a, pA, pm)
                        psd_b = psB.tile([128, 1024], f32, tag="psb")
                        keep_pass(psd_b, pB, pm)
                        s2_combine(2, pm, psd_a, psd_b)
                    pend = (m, A_t, B_t)
                pm, pA, pB = pend
                psd_a = psB.tile([128, 1024], f32, tag="psb")
                keep_pass(psd_a, pA, pm)
                psd_b = psB.tile([128, 1024], f32, tag="psb")
                keep_pass(psd_b, pB, pm)
                s2_combine(2, pm, psd_a, psd_b)

    nc.compile()
    return nc


def kernel(I, p, radius):
    assert int(radius) == R
    I = np.ascontiguousarray(np.asarray(I, np.float32))
    p = np.ascontiguousarray(np.asarray(p, np.float32))
    B = I.shape[0]
    assert I.shape == (B, 1, H, W) and p.shape == (B, 3, H, W)

    if "nc" not in _cache:
        _cache["nc"] = _build()
    nc = _cache["nc"]

    from concourse.bass_utils import run_bass_kernel_spmd

    b7 = _band7_np()
    strips = _strips()
    bm_f = _bandm_np(*strips[0])
    bm_i = _bandm_np(*strips[1])
    bm_l = _bandm_np(*strips[-1])

    in_maps = []
    for i in range(B):
        in_maps.append(
            {
                "I": I[i, 0],
                "p": p[i],
                "band7": b7,
                "bandm_first": bm_f,
                "bandm_int": bm_i,
                "bandm_last": bm_l,
            }
        )
    res = run_bass_kernel_spmd(nc, in_maps, core_ids=list(range(B)))
    out = np.stack([res.results[i]["q"] for i in range(B)], axis=0)
    return out.astype(np.float32)
